# revision 16
# baseline (speedup 1.0000x reference)
"""Trainium2 Bass kernel for Luong bilinear attention.

  out = softmax((q @ w) @ k^T) @ v      q:[B,Lq,Din] k,v:[B,Lk,Dout] w:[Din,Dout]

Sharding: 8 cores = 4 batches x 2 halves of Lq (data-parallel over batch,
sequence-parallel over Lq). k, v are replicated across the 2 cores of a batch.

Per-core layout strategy: scores are computed transposed, sT[k, q], so the
softmax denominator and the attention*V product are both plain matmuls with
k as the contraction (partition) dim:
    wqT[o, q] = w[i,o]^T . qT[i, q]          (PE, fp16)
    sT[k, q]  = kT[o, k]^T . wqT[o, q]       (PE, fp16, f32 PSUM)
    p[k, q]   = exp(sT)                      (ScalarE, f32 -> bf16)
    acc[q, 0:257] = p^T . [v | ones]         (PE; col 256 = softmax denom)
    out[q, o] = acc[:, 0:256] * (1/acc[:, 256])   (DVE)
exp() is applied without max-subtraction: scores ~ N(0, 12.8), |s| < ~70,
exp stays comfortably inside f32/bf16 range, and softmax is shift-invariant.

All device inputs are host-marshalled plane-major (partition dim first,
128-row planes folded into the free axis) so every input load is a single
2D DMA_DIRECT2D: the previous "(t p) x" rearranged loads each lowered to two
chained DMA instructions (the 2nd gated on the 1st's completion), which
serialized the startup load phase. Load priority order on the sync queue:
w -> qT -> kT (low-k halves of both planes first) -> v, so warmup/wq/scores
start as early as the HBM stream permits and scores(0) is never gated.

The av(qc-1) accumulation groups are interleaved into the scores(qc)
instruction stream explicitly. The PE executes its stream in order, so
without this the score matmuls race ahead of ScalarE's exp drain and stall
on PSUM score-buffer rotation (bufs=3) while av work sits queued behind
them. Placements leave >4us from block start before the first av group so
exp(qc-1)'s tail tiles are always drained.
"""

import numpy as np

B, LQ, LK, DIN, DOUT = 4, 4096, 4096, 256, 256
N_CORES = 8
QS = LQ // (N_CORES // B)  # 2048 queries per core
QC = 512                   # q-chunk (matmul free dim)
NQC = QS // QC             # 4 chunks
NKT = LK // 128            # 32 k tiles
VN = DOUT + 1              # v plus ones column

_prog_cache: dict = {}


def build_program(repeat: int = 1):
    """Build the (SPMD-identical) per-core Bass program."""
    if repeat in _prog_cache:
        return _prog_cache[repeat]
    from contextlib import ExitStack

    import concourse.bacc as bacc
    import concourse.mybir as mybir
    import concourse.tile as tile

    BF16 = mybir.dt.bfloat16
    FP16 = mybir.dt.float16
    F32 = mybir.dt.float32
    EXP = mybir.ActivationFunctionType.Exp

    nc = bacc.Bacc(
        "TRN2", target_bir_lowering=False, debug=False, num_devices=N_CORES
    )
    # all inputs host-marshalled plane-major: partition p in [0,128) first
    w_d = nc.dram_tensor("w", [128, 2, DOUT], FP16, kind="ExternalInput")
    # qT column-blocked: [partition, q-chunk, plane, q-within-chunk] so each
    # chunk is one contiguous-per-partition DMA and wq chunk i starts as
    # soon as chunk i lands -- the cold wq matmuls double as the HAM warmup
    qT_d = nc.dram_tensor("qT", [128, NQC, 2, QC], FP16, kind="ExternalInput")
    kT0_d = nc.dram_tensor("kT0", [128, LK], FP16, kind="ExternalInput")
    kT1_d = nc.dram_tensor("kT1", [128, LK], FP16, kind="ExternalInput")
    v_d = nc.dram_tensor("v", [128, NKT, DOUT], BF16, kind="ExternalInput")
    o_d = nc.dram_tensor("o", [QS, DOUT], F32, kind="ExternalOutput")

    with tile.TileContext(nc) as tc, ExitStack() as ctx:
        persist = ctx.enter_context(tc.tile_pool(name="persist", bufs=1))
        pexp = ctx.enter_context(tc.tile_pool(name="pexp", bufs=2))
        ps_pool = ctx.enter_context(
            tc.tile_pool(name="ps", bufs=3, space="PSUM")
        )
        po_pool = ctx.enter_context(
            tc.tile_pool(name="po", bufs=2, space="PSUM")
        )
        outp = ctx.enter_context(tc.tile_pool(name="outp", bufs=4))

        for _ in range(repeat):
            # ---- loads: one DMA_DIRECT2D each, strict priority order; each
            # ---- tensor split so several transfers are in flight at once
            w_bf = persist.tile([128, 2, DOUT], FP16, tag="w_bf")
            nc.sync.dma_start(w_bf[:], w_d.ap())

            qT_bf = persist.tile([128, NQC, 2, QC], FP16, tag="qT_bf")
            for qq in range(NQC):
                nc.sync.dma_start(qT_bf[:, qq, :, :], qT_d.ap()[:, qq, :, :])

            kt0_bf = persist.tile([128, LK], FP16, tag="kt0_bf")
            kt1_bf = persist.tile([128, LK], FP16, tag="kt1_bf")
            kt_bf = [kt0_bf, kt1_bf]
            HK = LK // 2
            nc.sync.dma_start(kt_bf[0][:, 0:HK], kT0_d.ap()[:, 0:HK])
            nc.sync.dma_start(kt_bf[1][:, 0:HK], kT1_d.ap()[:, 0:HK])
            nc.sync.dma_start(kt_bf[0][:, HK:LK], kT0_d.ap()[:, HK:LK])
            nc.sync.dma_start(kt_bf[1][:, HK:LK], kT1_d.ap()[:, HK:LK])

            # v: contiguous load into a staging tile (big DMA descriptors),
            # then DVE copy into the [v | ones] layout the av matmuls read.
            # A direct strided load has 512B descriptors whose generation and
            # draining throttles the whole load phase.
            v_raw = persist.tile([128, NKT, DOUT], BF16, tag="v_raw")
            HN = NKT // 2
            nc.sync.dma_start(v_raw[:, 0:HN, :], v_d.ap()[:, 0:HN, :])
            nc.sync.dma_start(v_raw[:, HN:NKT, :], v_d.ap()[:, HN:NKT, :])
            v_bf = persist.tile([128, NKT, VN], BF16, tag="v_bf")
            nc.vector.memset(v_bf[:, :, DOUT : DOUT + 1], 1.0)
            nc.vector.tensor_copy(v_bf[:, 0:HN, 0:DOUT], v_raw[:, 0:HN, :])
            nc.vector.tensor_copy(v_bf[:, HN:NKT, 0:DOUT], v_raw[:, HN:NKT, :])

            # ---- wqT[o, q] = w^T . qT (chunk qq gated on qT chunk qq; the
            # ---- cold first chunks double as HAM warmup) ----
            wq_bf = persist.tile([128, 2, QS], FP16, tag="wq_bf")
            for qq in range(NQC):
                for ot in range(2):
                    ps = ps_pool.tile([128, 2, QC], F32, tag="ps")
                    for it in range(2):
                        nc.tensor.matmul(
                            ps[:, 0, :],
                            w_bf[:, it, ot * 128 : (ot + 1) * 128],
                            qT_bf[:, qq, it, :],
                            start=(it == 0),
                            stop=(it == 1),
                        )
                    nc.vector.tensor_copy(
                        wq_bf[:, ot, qq * QC : (qq + 1) * QC], ps[:, 0, :]
                    )

            # ---- main loop ----
            def scores_ktg(qc, ktg, p_all):
                ps = ps_pool.tile([128, 2, QC], F32, tag="ps")
                for j in range(2):
                    kt = ktg * 2 + j
                    for it in range(2):
                        nc.tensor.matmul(
                            ps[:, j, :],
                            kt_bf[it][:, kt * 128 : (kt + 1) * 128],
                            wq_bf[:, it, qc * QC : (qc + 1) * QC],
                            start=(it == 0),
                            stop=(it == 1),
                        )
                nc.scalar.activation(
                    p_all[:, ktg * 2 : (ktg + 1) * 2, :], ps[:, :, :], EXP
                )

            def av_group(qc, qt, p_all):
                po = po_pool.tile([128, VN], F32, tag="po")
                for kt in range(NKT):
                    nc.tensor.matmul(
                        po[:],
                        p_all[:, kt, qt * 128 : (qt + 1) * 128],
                        v_bf[:, kt, :],
                        start=(kt == 0),
                        stop=(kt == NKT - 1),
                    )
                rec = outp.tile([128, 1], F32, tag="rec")
                nc.vector.reciprocal(rec[:], po[:, DOUT : DOUT + 1])
                o_sb = outp.tile([128, DOUT], F32, tag="o_sb")
                nc.vector.tensor_scalar_mul(o_sb[:], po[:, 0:DOUT], rec[:])
                r0 = (qc * (QC // 128) + qt) * 128
                nc.sync.dma_start(o_d.ap()[r0 : r0 + 128, :], o_sb[:])

            NGRP = QC // 128  # av groups per q-chunk
            # av(qc-1) group i is emitted after scores ktg av_slot[i];
            # slot None means after the whole scores block
            av_slots = {4: 0, 8: 1, 12: 2}

            p_tiles = []
            for qc in range(NQC):
                p_all = pexp.tile([128, NKT, QC], BF16, tag="p_all")
                p_tiles.append(p_all)
                for ktg in range(NKT // 2):
                    scores_ktg(qc, ktg, p_all)
                    if qc > 0 and ktg in av_slots:
                        av_group(qc - 1, av_slots[ktg], p_tiles[qc - 1])
                if qc > 0:
                    av_group(qc - 1, NGRP - 1, p_tiles[qc - 1])
            for qt in range(NGRP):
                av_group(NQC - 1, qt, p_tiles[NQC - 1])

    nc.compile()
    _prog_cache[repeat] = nc
    return nc


def make_in_maps(q, k, v, w):
    """Shard + marshal full inputs into per-core input maps.

    Marshalling includes the transpose of q/k, the rounding to the kernel's
    compute dtypes (fp16 score path, bf16 values), and the plane-major
    [128, ...] layouts the single-shot DMAs expect.
    """
    import ml_dtypes

    q = np.asarray(q, dtype=np.float32)
    k = np.asarray(k, dtype=np.float32)
    v = np.asarray(v, dtype=np.float32)
    w16 = (
        np.asarray(w, dtype=np.float32)
        .astype(np.float16)
        .reshape(2, 128, DOUT)
        .transpose(1, 0, 2)
        .copy()
    )
    kTp = []  # per batch: [2][128, LK] fp16 planes
    vp = []   # per batch: [128, NKT, DOUT] bf16
    for b in range(B):
        kT = k[b].T.astype(np.float16).reshape(2, 128, LK)
        kTp.append((kT[0].copy(), kT[1].copy()))
        vp.append(
            v[b]
            .astype(ml_dtypes.bfloat16)
            .reshape(NKT, 128, DOUT)
            .transpose(1, 0, 2)
            .copy()
        )
    in_maps = []
    for c in range(N_CORES):
        b, h = divmod(c, N_CORES // B)
        # [p, q-chunk, plane, q-within-chunk]
        qT = (
            q[b, h * QS : (h + 1) * QS, :]
            .T.astype(np.float16)
            .reshape(2, 128, NQC, QC)
            .transpose(1, 2, 0, 3)
            .copy()
        )
        in_maps.append(
            {
                "qT": qT,
                "kT0": kTp[b][0],
                "kT1": kTp[b][1],
                "v": vp[b],
                "w": w16,
            }
        )
    return in_maps


def kernel(q, v, k, w):
    from concourse import bass_utils

    nc = build_program()
    in_maps = make_in_maps(q, k, v, w)
    res = bass_utils.run_bass_kernel_spmd(nc, in_maps, core_ids=list(range(N_CORES)))
    out = np.empty((B, LQ, DOUT), dtype=np.float32)
    for c in range(N_CORES):
        b, h = divmod(c, N_CORES // B)
        out[b, h * QS : (h + 1) * QS, :] = res.results[c]["o"]
    return out


# revision 20
# speedup vs baseline: 1.0045x; 1.0045x over previous
"""Trainium2 Bass kernel for Luong bilinear attention.

  out = softmax((q @ w) @ k^T) @ v      q:[B,Lq,Din] k,v:[B,Lk,Dout] w:[Din,Dout]

Sharding: 8 cores = 4 batches x 2 halves of Lq (data-parallel over batch,
sequence-parallel over Lq). k, v are replicated across the 2 cores of a batch.

Per-core layout strategy: scores are computed transposed, sT[k, q], so the
softmax denominator and the attention*V product are both plain matmuls with
k as the contraction (partition) dim:
    wqT[o, q] = w[i,o]^T . qT[i, q]          (PE, fp16)
    sT[k, q]  = kT[o, k]^T . wqT[o, q]       (PE, fp16, f32 PSUM)
    p[k, q]   = exp(sT)                      (ScalarE, f32 -> bf16)
    acc[q, 0:257] = p^T . [v | ones]         (PE; col 256 = softmax denom)
    out[q, o] = acc[:, 0:256] * (1/acc[:, 256])   (DVE)
exp() is applied without max-subtraction: scores ~ N(0, 12.8), |s| < ~70,
exp stays comfortably inside f32/bf16 range, and softmax is shift-invariant.

All device inputs are host-marshalled plane-major (partition dim first,
128-row planes folded into the free axis) so every input load is a single
2D DMA_DIRECT2D: the previous "(t p) x" rearranged loads each lowered to two
chained DMA instructions (the 2nd gated on the 1st's completion), which
serialized the startup load phase. Load priority order on the sync queue:
w -> qT -> kT (low-k halves of both planes first) -> v, so warmup/wq/scores
start as early as the HBM stream permits and scores(0) is never gated.

The av(qc-1) accumulation groups are interleaved into the scores(qc)
instruction stream explicitly. The PE executes its stream in order, so
without this the score matmuls race ahead of ScalarE's exp drain and stall
on PSUM score-buffer rotation (bufs=3) while av work sits queued behind
them. Placements leave >4us from block start before the first av group so
exp(qc-1)'s tail tiles are always drained.
"""

import numpy as np

B, LQ, LK, DIN, DOUT = 4, 4096, 4096, 256, 256
N_CORES = 8
QS = LQ // (N_CORES // B)  # 2048 queries per core
QC = 512                   # q-chunk (matmul free dim)
NQC = QS // QC             # 4 chunks
NKT = LK // 128            # 32 k tiles
VN = DOUT + 1              # v plus ones column

_prog_cache: dict = {}


def build_program(repeat: int = 1):
    """Build the (SPMD-identical) per-core Bass program."""
    if repeat in _prog_cache:
        return _prog_cache[repeat]
    from contextlib import ExitStack

    import concourse.bacc as bacc
    import concourse.mybir as mybir
    import concourse.tile as tile

    BF16 = mybir.dt.bfloat16
    FP16 = mybir.dt.float16
    F32 = mybir.dt.float32
    EXP = mybir.ActivationFunctionType.Exp

    nc = bacc.Bacc(
        "TRN2", target_bir_lowering=False, debug=False, num_devices=N_CORES
    )
    # all inputs host-marshalled plane-major: partition p in [0,128) first
    w_d = nc.dram_tensor("w", [128, 2, DOUT], FP16, kind="ExternalInput")
    # qT column-blocked: [partition, q-half, plane, q-within-half] so each
    # half is one contiguous-per-partition DMA and wq pair h starts as soon
    # as half h lands (smaller pieces pay too much first-byte+receipt
    # latency and delay the kT stream behind them)
    qT_d = nc.dram_tensor("qT", [128, 2, 2, QS // 2], FP16, kind="ExternalInput")
    kT0_d = nc.dram_tensor("kT0", [128, LK], FP16, kind="ExternalInput")
    kT1_d = nc.dram_tensor("kT1", [128, LK], FP16, kind="ExternalInput")
    v_d = nc.dram_tensor("v", [128, NKT, DOUT], BF16, kind="ExternalInput")
    o_d = nc.dram_tensor("o", [QS, DOUT], F32, kind="ExternalOutput")

    with tile.TileContext(nc) as tc, ExitStack() as ctx:
        persist = ctx.enter_context(tc.tile_pool(name="persist", bufs=1))
        pexp = ctx.enter_context(tc.tile_pool(name="pexp", bufs=2))
        ps_pool = ctx.enter_context(
            tc.tile_pool(name="ps", bufs=3, space="PSUM")
        )
        po_pool = ctx.enter_context(
            tc.tile_pool(name="po", bufs=2, space="PSUM")
        )
        outp = ctx.enter_context(tc.tile_pool(name="outp", bufs=4))

        NWARM = 7  # PE warm-up matmuls on a DVE-memset tile (no DMA dep):
        # sustained PE activity from ~8us flips the HAM clock gate to full
        # rate right as the real wq matmuls begin
        for _ in range(repeat):
            wu = persist.tile([128, 2, DOUT], FP16, tag="wu")
            nc.vector.memset(wu[:], 0.0)
            wps = ps_pool.tile([128, 2, QC], F32, tag="ps")
            for _i in range(NWARM):
                nc.tensor.matmul(
                    wps[:, 0, :], wu[:, 0, 0:128], wu[:, :, :],
                    start=True, stop=True,
                )

            # ---- loads: one DMA_DIRECT2D each, strict priority order; each
            # ---- tensor split so several transfers are in flight at once
            w_bf = persist.tile([128, 2, DOUT], FP16, tag="w_bf")
            nc.sync.dma_start(w_bf[:], w_d.ap())

            qT_bf = persist.tile([128, 2, 2, QS // 2], FP16, tag="qT_bf")
            nc.sync.dma_start(qT_bf[:, 0, :, :], qT_d.ap()[:, 0, :, :])
            nc.sync.dma_start(qT_bf[:, 1, :, :], qT_d.ap()[:, 1, :, :])

            kt0_bf = persist.tile([128, LK], FP16, tag="kt0_bf")
            kt1_bf = persist.tile([128, LK], FP16, tag="kt1_bf")
            kt_bf = [kt0_bf, kt1_bf]
            HK = LK // 2
            nc.sync.dma_start(kt_bf[0][:, 0:HK], kT0_d.ap()[:, 0:HK])
            nc.sync.dma_start(kt_bf[1][:, 0:HK], kT1_d.ap()[:, 0:HK])
            nc.sync.dma_start(kt_bf[0][:, HK:LK], kT0_d.ap()[:, HK:LK])
            nc.sync.dma_start(kt_bf[1][:, HK:LK], kT1_d.ap()[:, HK:LK])

            # v: contiguous load into a staging tile (big DMA descriptors),
            # then DVE copy into the [v | ones] layout the av matmuls read.
            # A direct strided load has 512B descriptors whose generation and
            # draining throttles the whole load phase.
            v_raw = persist.tile([128, NKT, DOUT], BF16, tag="v_raw")
            HN = NKT // 2
            nc.sync.dma_start(v_raw[:, 0:HN, :], v_d.ap()[:, 0:HN, :])
            nc.sync.dma_start(v_raw[:, HN:NKT, :], v_d.ap()[:, HN:NKT, :])
            v_bf = persist.tile([128, NKT, VN], BF16, tag="v_bf")
            nc.vector.memset(v_bf[:, :, DOUT : DOUT + 1], 1.0)
            nc.vector.tensor_copy(v_bf[:, 0:HN, 0:DOUT], v_raw[:, 0:HN, :])
            nc.vector.tensor_copy(v_bf[:, HN:NKT, 0:DOUT], v_raw[:, HN:NKT, :])

            # ---- wqT[o, q] = w^T . qT (pair qc2 gated on qT half qc2) ----
            wq_bf = persist.tile([128, 2, QS], FP16, tag="wq_bf")
            for qc2 in range(NQC // 2):
                for ot in range(2):
                    ps = ps_pool.tile([128, 2, QC], F32, tag="ps")
                    for j in range(2):
                        for it in range(2):
                            nc.tensor.matmul(
                                ps[:, j, :],
                                w_bf[:, it, ot * 128 : (ot + 1) * 128],
                                qT_bf[:, qc2, it, j * QC : (j + 1) * QC],
                                start=(it == 0),
                                stop=(it == 1),
                            )
                    nc.vector.tensor_copy(
                        wq_bf[:, ot, qc2 * 2 * QC : (qc2 + 1) * 2 * QC],
                        ps[:, :, :],
                    )

            # ---- main loop ----
            def scores_ktg(qc, ktg, p_all):
                ps = ps_pool.tile([128, 2, QC], F32, tag="ps")
                for j in range(2):
                    kt = ktg * 2 + j
                    for it in range(2):
                        nc.tensor.matmul(
                            ps[:, j, :],
                            kt_bf[it][:, kt * 128 : (kt + 1) * 128],
                            wq_bf[:, it, qc * QC : (qc + 1) * QC],
                            start=(it == 0),
                            stop=(it == 1),
                        )
                nc.scalar.activation(
                    p_all[:, ktg * 2 : (ktg + 1) * 2, :], ps[:, :, :], EXP
                )

            def av_group(qc, qt, p_all):
                po = po_pool.tile([128, VN], F32, tag="po")
                for kt in range(NKT):
                    nc.tensor.matmul(
                        po[:],
                        p_all[:, kt, qt * 128 : (qt + 1) * 128],
                        v_bf[:, kt, :],
                        start=(kt == 0),
                        stop=(kt == NKT - 1),
                    )
                rec = outp.tile([128, 1], F32, tag="rec")
                nc.vector.reciprocal(rec[:], po[:, DOUT : DOUT + 1])
                o_sb = outp.tile([128, DOUT], F32, tag="o_sb")
                nc.vector.tensor_scalar_mul(o_sb[:], po[:, 0:DOUT], rec[:])
                r0 = (qc * (QC // 128) + qt) * 128
                nc.sync.dma_start(o_d.ap()[r0 : r0 + 128, :], o_sb[:])

            NGRP = QC // 128  # av groups per q-chunk
            # av(qc-1) group i is emitted after scores ktg av_slot[i];
            # slot None means after the whole scores block
            av_slots = {4: 0, 8: 1, 12: 2}

            p_tiles = []
            for qc in range(NQC):
                p_all = pexp.tile([128, NKT, QC], BF16, tag="p_all")
                p_tiles.append(p_all)
                for ktg in range(NKT // 2):
                    scores_ktg(qc, ktg, p_all)
                    if qc > 0 and ktg in av_slots:
                        av_group(qc - 1, av_slots[ktg], p_tiles[qc - 1])
                if qc > 0:
                    av_group(qc - 1, NGRP - 1, p_tiles[qc - 1])
            for qt in range(NGRP):
                av_group(NQC - 1, qt, p_tiles[NQC - 1])

    nc.compile()
    _prog_cache[repeat] = nc
    return nc


def make_in_maps(q, k, v, w):
    """Shard + marshal full inputs into per-core input maps.

    Marshalling includes the transpose of q/k, the rounding to the kernel's
    compute dtypes (fp16 score path, bf16 values), and the plane-major
    [128, ...] layouts the single-shot DMAs expect.
    """
    import ml_dtypes

    q = np.asarray(q, dtype=np.float32)
    k = np.asarray(k, dtype=np.float32)
    v = np.asarray(v, dtype=np.float32)
    w16 = (
        np.asarray(w, dtype=np.float32)
        .astype(np.float16)
        .reshape(2, 128, DOUT)
        .transpose(1, 0, 2)
        .copy()
    )
    kTp = []  # per batch: [2][128, LK] fp16 planes
    vp = []   # per batch: [128, NKT, DOUT] bf16
    for b in range(B):
        kT = k[b].T.astype(np.float16).reshape(2, 128, LK)
        kTp.append((kT[0].copy(), kT[1].copy()))
        vp.append(
            v[b]
            .astype(ml_dtypes.bfloat16)
            .reshape(NKT, 128, DOUT)
            .transpose(1, 0, 2)
            .copy()
        )
    in_maps = []
    for c in range(N_CORES):
        b, h = divmod(c, N_CORES // B)
        # [p, q-half, plane, q-within-half]
        qT = (
            q[b, h * QS : (h + 1) * QS, :]
            .T.astype(np.float16)
            .reshape(2, 128, 2, QS // 2)
            .transpose(1, 2, 0, 3)
            .copy()
        )
        in_maps.append(
            {
                "qT": qT,
                "kT0": kTp[b][0],
                "kT1": kTp[b][1],
                "v": vp[b],
                "w": w16,
            }
        )
    return in_maps


def kernel(q, v, k, w):
    from concourse import bass_utils

    nc = build_program()
    in_maps = make_in_maps(q, k, v, w)
    res = bass_utils.run_bass_kernel_spmd(nc, in_maps, core_ids=list(range(N_CORES)))
    out = np.empty((B, LQ, DOUT), dtype=np.float32)
    for c in range(N_CORES):
        b, h = divmod(c, N_CORES // B)
        out[b, h * QS : (h + 1) * QS, :] = res.results[c]["o"]
    return out


# revision 21
# speedup vs baseline: 1.0157x; 1.0111x over previous
"""Trainium2 Bass kernel for Luong bilinear attention.

  out = softmax((q @ w) @ k^T) @ v      q:[B,Lq,Din] k,v:[B,Lk,Dout] w:[Din,Dout]

Sharding: 8 cores = 4 batches x 2 halves of Lq (data-parallel over batch,
sequence-parallel over Lq). k, v are replicated across the 2 cores of a batch.

Per-core layout strategy: scores are computed transposed, sT[k, q], so the
softmax denominator and the attention*V product are both plain matmuls with
k as the contraction (partition) dim:
    wqT[o, q] = w[i,o]^T . qT[i, q]          (PE, fp16)
    sT[k, q]  = kT[o, k]^T . wqT[o, q]       (PE, fp16, f32 PSUM)
    p[k, q]   = exp(sT)                      (ScalarE, f32 -> bf16)
    acc[q, 0:257] = p^T . [v | ones]         (PE; col 256 = softmax denom)
    out[q, o] = acc[:, 0:256] * (1/acc[:, 256])   (DVE)
exp() is applied without max-subtraction: scores ~ N(0, 12.8), |s| < ~70,
exp stays comfortably inside f32/bf16 range, and softmax is shift-invariant.

All device inputs are host-marshalled plane-major (partition dim first,
128-row planes folded into the free axis) so every input load is a single
2D DMA_DIRECT2D: the previous "(t p) x" rearranged loads each lowered to two
chained DMA instructions (the 2nd gated on the 1st's completion), which
serialized the startup load phase. Load priority order on the sync queue:
w -> qT -> kT (low-k halves of both planes first) -> v, so warmup/wq/scores
start as early as the HBM stream permits and scores(0) is never gated.

The av(qc-1) accumulation groups are interleaved into the scores(qc)
instruction stream explicitly. The PE executes its stream in order, so
without this the score matmuls race ahead of ScalarE's exp drain and stall
on PSUM score-buffer rotation (bufs=3) while av work sits queued behind
them. Placements leave >4us from block start before the first av group so
exp(qc-1)'s tail tiles are always drained.
"""

import numpy as np

B, LQ, LK, DIN, DOUT = 4, 4096, 4096, 256, 256
N_CORES = 8
QS = LQ // (N_CORES // B)  # 2048 queries per core
QC = 512                   # q-chunk (matmul free dim)
NQC = QS // QC             # 4 chunks
NKT = LK // 128            # 32 k tiles
VN = DOUT + 1              # v plus ones column

_prog_cache: dict = {}


def build_program(repeat: int = 1):
    """Build the (SPMD-identical) per-core Bass program."""
    if repeat in _prog_cache:
        return _prog_cache[repeat]
    from contextlib import ExitStack

    import concourse.bacc as bacc
    import concourse.mybir as mybir
    import concourse.tile as tile

    BF16 = mybir.dt.bfloat16
    FP16 = mybir.dt.float16
    F32 = mybir.dt.float32
    EXP = mybir.ActivationFunctionType.Exp

    nc = bacc.Bacc(
        "TRN2", target_bir_lowering=False, debug=False, num_devices=N_CORES
    )
    # all inputs host-marshalled plane-major: partition p in [0,128) first
    w_d = nc.dram_tensor("w", [128, 2, DOUT], FP16, kind="ExternalInput")
    # qT column-blocked: [partition, q-half, plane, q-within-half] so each
    # half is one contiguous-per-partition DMA and wq pair h starts as soon
    # as half h lands (smaller pieces pay too much first-byte+receipt
    # latency and delay the kT stream behind them)
    qT_d = nc.dram_tensor("qT", [128, 2, 2, QS // 2], FP16, kind="ExternalInput")
    kT0_d = nc.dram_tensor("kT0", [128, LK], FP16, kind="ExternalInput")
    kT1_d = nc.dram_tensor("kT1", [128, LK], FP16, kind="ExternalInput")
    v_d = nc.dram_tensor("v", [128, NKT, DOUT], BF16, kind="ExternalInput")
    o_d = nc.dram_tensor("o", [QS, DOUT], F32, kind="ExternalOutput")

    with tile.TileContext(nc) as tc, ExitStack() as ctx:
        persist = ctx.enter_context(tc.tile_pool(name="persist", bufs=1))
        pexp = ctx.enter_context(tc.tile_pool(name="pexp", bufs=2))
        ps_pool = ctx.enter_context(
            tc.tile_pool(name="ps", bufs=3, space="PSUM")
        )
        po_pool = ctx.enter_context(
            tc.tile_pool(name="po", bufs=2, space="PSUM")
        )
        outp = ctx.enter_context(tc.tile_pool(name="outp", bufs=4))

        NWARM = 12  # PE warm-up matmuls on a DVE-memset tile (no DMA dep):
        # sustained PE activity from ~8us flips the HAM clock gate (~3.4us)
        # and keeps the PE busy across the whole qT/kT load window, so every
        # real matmul runs at full clock with its data resident
        for _ in range(repeat):
            wu = persist.tile([128, 2, DOUT], FP16, tag="wu")
            nc.vector.memset(wu[:], 0.0)
            wps = ps_pool.tile([128, 2, QC], F32, tag="ps")
            for _i in range(NWARM):
                nc.tensor.matmul(
                    wps[:, 0, :], wu[:, 0, 0:128], wu[:, :, :],
                    start=True, stop=True,
                )

            # ---- loads: one DMA_DIRECT2D each, strict priority order; each
            # ---- tensor split so several transfers are in flight at once
            w_bf = persist.tile([128, 2, DOUT], FP16, tag="w_bf")
            nc.sync.dma_start(w_bf[:], w_d.ap())

            qT_bf = persist.tile([128, 2, 2, QS // 2], FP16, tag="qT_bf")
            nc.sync.dma_start(qT_bf[:, 0, :, :], qT_d.ap()[:, 0, :, :])
            nc.sync.dma_start(qT_bf[:, 1, :, :], qT_d.ap()[:, 1, :, :])

            kt0_bf = persist.tile([128, LK], FP16, tag="kt0_bf")
            kt1_bf = persist.tile([128, LK], FP16, tag="kt1_bf")
            kt_bf = [kt0_bf, kt1_bf]
            HK = LK // 2
            nc.sync.dma_start(kt_bf[0][:, 0:HK], kT0_d.ap()[:, 0:HK])
            nc.sync.dma_start(kt_bf[1][:, 0:HK], kT1_d.ap()[:, 0:HK])
            nc.sync.dma_start(kt_bf[0][:, HK:LK], kT0_d.ap()[:, HK:LK])
            nc.sync.dma_start(kt_bf[1][:, HK:LK], kT1_d.ap()[:, HK:LK])

            # v: contiguous load into a staging tile (big DMA descriptors),
            # then DVE copy into the [v | ones] layout the av matmuls read.
            # A direct strided load has 512B descriptors whose generation and
            # draining throttles the whole load phase.
            v_raw = persist.tile([128, NKT, DOUT], BF16, tag="v_raw")
            HN = NKT // 2
            nc.sync.dma_start(v_raw[:, 0:HN, :], v_d.ap()[:, 0:HN, :])
            nc.sync.dma_start(v_raw[:, HN:NKT, :], v_d.ap()[:, HN:NKT, :])
            v_bf = persist.tile([128, NKT, VN], BF16, tag="v_bf")
            nc.vector.memset(v_bf[:, :, DOUT : DOUT + 1], 1.0)
            nc.vector.tensor_copy(v_bf[:, 0:HN, 0:DOUT], v_raw[:, 0:HN, :])
            nc.vector.tensor_copy(v_bf[:, HN:NKT, 0:DOUT], v_raw[:, HN:NKT, :])

            # ---- wqT[o, q] = w^T . qT (pair qc2 gated on qT half qc2) ----
            wq_bf = persist.tile([128, 2, QS], FP16, tag="wq_bf")
            for qc2 in range(NQC // 2):
                for ot in range(2):
                    ps = ps_pool.tile([128, 2, QC], F32, tag="ps")
                    for j in range(2):
                        for it in range(2):
                            nc.tensor.matmul(
                                ps[:, j, :],
                                w_bf[:, it, ot * 128 : (ot + 1) * 128],
                                qT_bf[:, qc2, it, j * QC : (j + 1) * QC],
                                start=(it == 0),
                                stop=(it == 1),
                            )
                    nc.vector.tensor_copy(
                        wq_bf[:, ot, qc2 * 2 * QC : (qc2 + 1) * 2 * QC],
                        ps[:, :, :],
                    )

            # ---- main loop ----
            def scores_ktg(qc, ktg, p_all):
                ps = ps_pool.tile([128, 2, QC], F32, tag="ps")
                for j in range(2):
                    kt = ktg * 2 + j
                    for it in range(2):
                        nc.tensor.matmul(
                            ps[:, j, :],
                            kt_bf[it][:, kt * 128 : (kt + 1) * 128],
                            wq_bf[:, it, qc * QC : (qc + 1) * QC],
                            start=(it == 0),
                            stop=(it == 1),
                        )
                nc.scalar.activation(
                    p_all[:, ktg * 2 : (ktg + 1) * 2, :], ps[:, :, :], EXP
                )

            def av_group(qc, qt, p_all):
                po = po_pool.tile([128, VN], F32, tag="po")
                for kt in range(NKT):
                    nc.tensor.matmul(
                        po[:],
                        p_all[:, kt, qt * 128 : (qt + 1) * 128],
                        v_bf[:, kt, :],
                        start=(kt == 0),
                        stop=(kt == NKT - 1),
                    )
                rec = outp.tile([128, 1], F32, tag="rec")
                nc.vector.reciprocal(rec[:], po[:, DOUT : DOUT + 1])
                o_sb = outp.tile([128, DOUT], F32, tag="o_sb")
                nc.vector.tensor_scalar_mul(o_sb[:], po[:, 0:DOUT], rec[:])
                r0 = (qc * (QC // 128) + qt) * 128
                nc.sync.dma_start(o_d.ap()[r0 : r0 + 128, :], o_sb[:])

            NGRP = QC // 128  # av groups per q-chunk
            # av(qc-1) group i is emitted after scores ktg av_slot[i];
            # slot None means after the whole scores block
            av_slots = {4: 0, 8: 1, 12: 2}

            p_tiles = []
            for qc in range(NQC):
                p_all = pexp.tile([128, NKT, QC], BF16, tag="p_all")
                p_tiles.append(p_all)
                for ktg in range(NKT // 2):
                    scores_ktg(qc, ktg, p_all)
                    if qc > 0 and ktg in av_slots:
                        av_group(qc - 1, av_slots[ktg], p_tiles[qc - 1])
                if qc > 0:
                    av_group(qc - 1, NGRP - 1, p_tiles[qc - 1])
            for qt in range(NGRP):
                av_group(NQC - 1, qt, p_tiles[NQC - 1])

    nc.compile()
    _prog_cache[repeat] = nc
    return nc


def make_in_maps(q, k, v, w):
    """Shard + marshal full inputs into per-core input maps.

    Marshalling includes the transpose of q/k, the rounding to the kernel's
    compute dtypes (fp16 score path, bf16 values), and the plane-major
    [128, ...] layouts the single-shot DMAs expect.
    """
    import ml_dtypes

    q = np.asarray(q, dtype=np.float32)
    k = np.asarray(k, dtype=np.float32)
    v = np.asarray(v, dtype=np.float32)
    w16 = (
        np.asarray(w, dtype=np.float32)
        .astype(np.float16)
        .reshape(2, 128, DOUT)
        .transpose(1, 0, 2)
        .copy()
    )
    kTp = []  # per batch: [2][128, LK] fp16 planes
    vp = []   # per batch: [128, NKT, DOUT] bf16
    for b in range(B):
        kT = k[b].T.astype(np.float16).reshape(2, 128, LK)
        kTp.append((kT[0].copy(), kT[1].copy()))
        vp.append(
            v[b]
            .astype(ml_dtypes.bfloat16)
            .reshape(NKT, 128, DOUT)
            .transpose(1, 0, 2)
            .copy()
        )
    in_maps = []
    for c in range(N_CORES):
        b, h = divmod(c, N_CORES // B)
        # [p, q-half, plane, q-within-half]
        qT = (
            q[b, h * QS : (h + 1) * QS, :]
            .T.astype(np.float16)
            .reshape(2, 128, 2, QS // 2)
            .transpose(1, 2, 0, 3)
            .copy()
        )
        in_maps.append(
            {
                "qT": qT,
                "kT0": kTp[b][0],
                "kT1": kTp[b][1],
                "v": vp[b],
                "w": w16,
            }
        )
    return in_maps


def kernel(q, v, k, w):
    from concourse import bass_utils

    nc = build_program()
    in_maps = make_in_maps(q, k, v, w)
    res = bass_utils.run_bass_kernel_spmd(nc, in_maps, core_ids=list(range(N_CORES)))
    out = np.empty((B, LQ, DOUT), dtype=np.float32)
    for c in range(N_CORES):
        b, h = divmod(c, N_CORES // B)
        out[b, h * QS : (h + 1) * QS, :] = res.results[c]["o"]
    return out
